# revision 10
# baseline (speedup 1.0000x reference)
"""Block-sparse position-wise FFN on Trainium2 (Bass/Tile), 8-core data-parallel.

Strategy:
  - Shard tokens (B*S = 36928) evenly across 8 cores: 4616 tokens/core.
    The FFN is pointwise over tokens and both (masked) weight matrices fit
    in SBUF, so data-parallel needs no collectives.
  - Host prep: apply the 8x8 block masks to W1/W2 (weights+masks are layer
    constants) and pre-transpose to the layouts the PE wants. x is fed in
    its natural [T, DIM] layout and transposed on device via PE-transpose.
  - Per core, fused loop over token chunks (<=511 tokens, PSUM-bank sized):
      xT = transpose(x_chunk)                  (PE transpose + DVE copy)
      h  = gelu(W1m @ xT + b1)                 (fp32r matmuls, ACT gelu+bias)
      out_chunk = (hT as stationary).T @ W2mT + b2   (natural-layout output)
    float32r matmul dtype streams at 1 cycle/row for free dim >= 256
    (plain float32 is 4 cycles/row).
"""

import numpy as np

import concourse.bass as bass
import concourse.bacc as bacc
import concourse.mybir as mybir
from concourse import tile, masks
from concourse.bass_utils import run_bass_kernel_spmd

B, S, DIM, FF, BLK = 64, 577, 768, 3072, 8
NCORES = 8
TOK = B * S                # 36928
T = TOK // NCORES          # 4616 tokens per core
P = 128
KD = DIM // P              # 6 k-tiles for fc1
KF = FF // P               # 24 f-tiles
F32 = mybir.dt.float32
F32R = mybir.dt.float32r
GELU = mybir.ActivationFunctionType.Gelu


def _chunks(total):
    """Token chunks: 256 wide until the tail (256..511) so every fc1 matmul
    keeps free dim >= 256 (fp32r full rate) and <= 511 (one PSUM bank)."""
    out, pos = [], 0
    while pos < total:
        w = 256 if total - pos >= 512 else total - pos
        out.append((pos, w))
        pos += w
    return out


def _token_tiles(w):
    tiles, off = [], 0
    while off < w:
        p = min(P, w - off)
        tiles.append((off, p))
        off += p
    return tiles


def _body(tc, x_d, w1_d, b1_d, w2_d, b2_d, o_d, t_tokens):
    nc = tc.nc
    with (
        tc.tile_pool(name="const", bufs=1) as constp,
        tc.tile_pool(name="wpool", bufs=1) as wp,
        tc.tile_pool(name="xnat", bufs=4) as xnatp,
        tc.tile_pool(name="xt", bufs=2) as xtp,
        tc.tile_pool(name="ht", bufs=26) as htp,
        tc.tile_pool(name="onat", bufs=2) as onatp,
        tc.tile_pool(name="pst", bufs=2, space=bass.MemorySpace.PSUM) as pstp,
        tc.tile_pool(name="ps1", bufs=2, space=bass.MemorySpace.PSUM) as ps1p,
        tc.tile_pool(name="ps2", bufs=2, space=bass.MemorySpace.PSUM) as ps2p,
    ):
        ident_f = constp.tile([P, P], F32)
        masks.make_identity(nc, ident_f[:])
        ident = constp.tile([P, P], F32R)
        nc.vector.tensor_copy(ident[:], ident_f[:])
        b1_s = constp.tile([P, KF], F32)
        nc.sync.dma_start(out=b1_s[:], in_=b1_d)
        b2_s = constp.tile([P, DIM], F32)
        nc.sync.dma_start(out=b2_s[:], in_=b2_d)

        w1_s = []
        for k in range(KD):
            w = wp.tile([P, FF], F32R, tag=f"w1_{k}", name=f"w1_{k}")
            w1_s.append(w)
        W1CHUNK = FF // 4
        for cc in range(4):
            for k in range(KD):
                nc.sync.dma_start(
                    out=w1_s[k][:, cc * W1CHUNK:(cc + 1) * W1CHUNK],
                    in_=w1_d[k * P:(k + 1) * P,
                             cc * W1CHUNK:(cc + 1) * W1CHUNK],
                )
        w2_s = []
        for k in range(KF):
            w = wp.tile([P, DIM], F32R, tag=f"w2_{k}")
            nc.sync.dma_start(out=w[:], in_=w2_d[k * P:(k + 1) * P, :])
            w2_s.append(w)

        chunks = _chunks(t_tokens)

        def load_transpose(c0, cw):
            """DMA a token chunk and PE-transpose it into xT tiles."""
            xts = [xtp.tile([P, cw], F32R, tag=f"xt{k}", name=f"xt{k}")
                   for k in range(KD)]
            for (toff, tp) in _token_tiles(cw):
                xn = xnatp.tile([P, DIM], F32R, tag="xn", name="xn")
                nc.scalar.dma_start(
                    out=xn[0:tp, :], in_=x_d[c0 + toff:c0 + toff + tp, :]
                )
                for k in range(KD):
                    pst = pstp.tile([P, P], F32R, tag="pst", name="pst")
                    nc.tensor.transpose(
                        pst[:, 0:tp], xn[0:tp, k * P:(k + 1) * P],
                        ident[0:tp, 0:tp],
                    )
                    nc.vector.tensor_copy(
                        xts[k][:, toff:toff + tp], pst[:, 0:tp]
                    )
            return xts

        xts = load_transpose(*chunks[0])
        for ci, (c0, cw) in enumerate(chunks):
            ttiles = _token_tiles(cw)

            # --- fc1: hT[m] = gelu(W1mT[:,m].T @ xT + b1[m]) ---
            hts = []
            for m in range(KF):
                ps1 = ps1p.tile([P, cw], F32, tag="ps1")
                for k in range(KD):
                    nc.tensor.matmul(
                        ps1[:, :],
                        w1_s[k][:, m * P:(m + 1) * P],
                        xts[k][:, :],
                        start=(k == 0), stop=(k == KD - 1),
                    )
                ht = htp.tile([P, cw], F32R, tag="ht")
                nc.scalar.activation(
                    ht[:, :], ps1[:, :], GELU, bias=b1_s[:, m:m + 1]
                )
                hts.append(ht)

            # --- prefetch+transpose next chunk while fc2 runs ---
            next_xts = (load_transpose(*chunks[ci + 1])
                        if ci + 1 < len(chunks) else None)

            # --- fc2: out[t, :] = hT.T @ W2mT + b2, natural layout ---
            for (toff, tp) in ttiles:
                ps2 = ps2p.tile([P, DIM], F32, tag="ps2")
                for k in range(KF):
                    last = (k == KF - 1)
                    for off, wdt in ((0, 512), (512, DIM - 512)):
                        nc.tensor.matmul(
                            ps2[0:tp, off:off + wdt],
                            hts[k][:, toff:toff + tp],
                            w2_s[k][:, off:off + wdt],
                            start=(k == 0), stop=last,
                        )
                on = onatp.tile([P, DIM], F32, tag="on")
                nc.vector.tensor_tensor(
                    out=on[0:tp, :], in0=ps2[0:tp, :], in1=b2_s[0:tp, :],
                    op=mybir.AluOpType.add,
                )
                nc.sync.dma_start(
                    out=o_d[c0 + toff:c0 + toff + tp, :], in_=on[0:tp, :]
                )
            xts = next_xts


def build_program(t_tokens=T):
    nc = bacc.Bacc("TRN2", target_bir_lowering=False, debug=False,
                   num_devices=NCORES)
    x_d = nc.dram_tensor("x", [t_tokens, DIM], F32R, kind="ExternalInput").ap()
    w1_d = nc.dram_tensor("w1t", [DIM, FF], F32R, kind="ExternalInput").ap()
    b1_d = nc.dram_tensor("b1", [P, KF], F32, kind="ExternalInput").ap()
    w2_d = nc.dram_tensor("w2t", [FF, DIM], F32R, kind="ExternalInput").ap()
    b2_d = nc.dram_tensor("b2", [P, DIM], F32, kind="ExternalInput").ap()
    o_d = nc.dram_tensor("out", [t_tokens, DIM], F32, kind="ExternalOutput").ap()
    with tile.TileContext(nc) as tc:
        _body(tc, x_d, w1_d, b1_d, w2_d, b2_d, o_d, t_tokens)
    nc.compile()
    return nc


def _round_fp32r(a):
    """Round fp32 values to the fp32r grid (low 12 mantissa bits dropped,
    round-to-nearest), matching the PE's fp32r operand rounding."""
    u = a.view(np.uint32)
    u = (u + np.uint32(0x800)) & np.uint32(0xFFFFF000)
    return u.view(np.float32)


def host_prep(x, W1, b1, W2, b2, mask1, mask2):
    x = _round_fp32r(np.ascontiguousarray(
        np.asarray(x, dtype=np.float32).reshape(TOK, DIM)))
    m1 = np.repeat(np.repeat(np.asarray(mask1, dtype=bool), BLK, 0), BLK, 1)
    m2 = np.repeat(np.repeat(np.asarray(mask2, dtype=bool), BLK, 0), BLK, 1)
    w1t = _round_fp32r(np.ascontiguousarray(
        (np.asarray(W1, np.float32) * m1.astype(np.float32)).T))  # [DIM, FF]
    w2t = _round_fp32r(np.ascontiguousarray(
        (np.asarray(W2, np.float32) * m2.astype(np.float32)).T))  # [FF, DIM]
    b1h = np.ascontiguousarray(
        np.asarray(b1, np.float32).reshape(KF, P).T)              # [P, KF]
    b2h = np.ascontiguousarray(
        np.broadcast_to(np.asarray(b2, np.float32)[None, :], (P, DIM)))
    return x, w1t, b1h, w2t, b2h


_PROGRAM = None


def _get_program():
    global _PROGRAM
    if _PROGRAM is None:
        _PROGRAM = build_program(T)
    return _PROGRAM


def kernel(x, W1, b1, W2, b2, mask1, mask2, **run_kwargs):
    xs, w1t, b1h, w2t, b2h = host_prep(x, W1, b1, W2, b2, mask1, mask2)
    nc = _get_program()
    in_maps = [
        {"x": xs[c * T:(c + 1) * T], "w1t": w1t, "b1": b1h,
         "w2t": w2t, "b2": b2h}
        for c in range(NCORES)
    ]
    res = run_bass_kernel_spmd(nc, in_maps, list(range(NCORES)), **run_kwargs)
    out = np.concatenate([res.results[c]["out"] for c in range(NCORES)], axis=0)
    out = out.reshape(B, S, DIM).astype(np.float32)
    if run_kwargs:
        kernel.last_results = res
    return out
